# revision 4
# baseline (speedup 1.0000x reference)
"""Trainium2 Bass kernel for AlignOnlySubLayer.

Per batch b:
    W[c,m]   = sum_d context[b,c,d] * main[b,m,d]
    A        = softmax(W, axis=m)
    out[m,d] = main[b,m,d] - sum_c A[c,m] * context[b,c,d]

Sharding: data-parallel over batch B=8 across the 8 NeuronCores (one batch
per core, no cross-core communication).

Kernel strategy (per core):
  - Both matmuls contract with d (or c) on the partition axis, so mm1 needs
    d-major (transposed) copies of context/main. The DMA xbar transpose only
    moves 2-byte dtypes, so each f32 matrix is split into an fp16 hi part and
    an fp16 residual lo part, both transposed, then recombined to f32 in the
    transposed layout (error ~1e-7 relative; f32 matmul runs at the same PE
    rate as fp16 so there is no throughput cost, only DVE/DMA prologue work).
  - mm1 (f32): W tiles [c=128, m=2048] into rotating PSUM half-buffers.
  - Softmax without max-subtraction (|W| <~ 70 << 88.7 fp32 exp overflow
    limit, and softmax is shift-invariant so results match the reference):
    ACT Exp reads PSUM directly with fused per-row accumulation giving the
    row sums S[c]; no separate reduce pass.
  - Normalization folded into context: ctx_s[c,:] = context[c,:] / S[c]
    (scales a 2048x128 matrix instead of the 2048x2048 weights).
  - mm2 (f32): weightedT[d,m] += ctx_s[c-chunk].T @ E[c-chunk] accumulated
    over all 16 c-chunks in a persistent 4-bank PSUM region, N=512 matmuls.
  - Tail: weightedT -> fp16 hi/lo, xbar-transpose back to [m,d], and
    out = ((main - hi) - lo) in f32, streamed per quarter.
"""

import numpy as np

import concourse.bass as bass
import concourse.mybir as mybir
from concourse import bacc
from concourse.tile import TileContext
from concourse.bass_utils import run_bass_kernel_spmd

P = 128
F32 = mybir.dt.float32
F16 = mybir.dt.float16
EXP = mybir.ActivationFunctionType.Exp
N_CORES = 8
TAIL_HILO = True


def build_nc(S=2048, D=128, num_devices=N_CORES, repeats=1):
    """Build the single-core Bass program (SPMD across cores).

    repeats > 1 unrolls the whole computation R times (same inputs/outputs)
    purely for wall-clock timing: per-iteration HW time is estimated from
    the difference between R=R and R=1 builds.
    """
    assert D == P and S % P == 0
    T = S // P            # number of 128-row tiles along c (and m)
    QT = max(1, T // 4)   # tiles per prologue/tail chunk
    NQ = T // QT          # number of chunks
    QW = QT * P           # columns per chunk
    HALF = S // 2         # columns per mm1 psum half

    nc = bacc.Bacc(
        "TRN2",
        target_bir_lowering=False,
        debug=False,
        enable_asserts=False,
        num_devices=num_devices,
    )
    ctx_d = nc.dram_tensor("context", [S, D], F32, kind="ExternalInput").ap()
    main_d = nc.dram_tensor("main", [S, D], F32, kind="ExternalInput").ap()
    out_d = nc.dram_tensor("out", [S, D], F32, kind="ExternalOutput").ap()

    ctx_dt = ctx_d.rearrange("(t p) d -> p t d", p=P)
    main_dt = main_d.rearrange("(t p) d -> p t d", p=P)
    out_dt = out_d.rearrange("(t p) d -> p t d", p=P)

    with TileContext(nc) as tc:
      for _rep in range(repeats):
        with (
            tc.tile_pool(name="persist", bufs=1) as persist,
            tc.tile_pool(name="prolog", bufs=2) as prolog,
            tc.tile_pool(name="etile", bufs=2) as etile_pool,
            tc.tile_pool(name="small", bufs=3) as small,
            tc.tile_pool(name="tailp", bufs=2) as tailp,
            tc.tile_pool(name="psum_w", bufs=2, space="PSUM") as psum_w,
            tc.tile_pool(name="psum_acc", bufs=1, space="PSUM") as psum_acc,
        ):
            # ---- persistent SBUF tensors ----
            ctx_f32 = persist.tile([P, T, P], F32)     # natural [c_in, ct, d]
            main_f32 = persist.tile([P, T, P], F32)    # natural [m_in, mt, d]
            ctxT = persist.tile([P, T, P], F32)        # [d, ct, c_in]
            mainT = persist.tile([P, T, P], F32)       # [d, mt, m_in]
            mainT2 = mainT.rearrange("p a b -> p (a b)")

            # Warm the ACT exp table early so the ~2.7us table load overlaps
            # the prologue DMAs.
            warm = small.tile([P, 1], F32, tag="warm")
            nc.vector.memset(warm[:], 0.0)
            nc.scalar.activation(warm[:], warm[:], EXP)

            # ---- prologue: load, hi/lo split, transpose, recombine ----
            def load_transposed(src_dt, nat_f32, dstT, q, mtag):
                ts = slice(q * QT, (q + 1) * QT)
                nc.gpsimd.dma_start(nat_f32[:, ts], src_dt[:, ts])
                nat2 = nat_f32[:, ts].rearrange("p a b -> p (a b)")
                hi = prolog.tile([P, QW], F16, tag=f"hi_{mtag}")
                lo = prolog.tile([P, QW], F16, tag=f"lo_{mtag}")
                nc.vector.tensor_copy(hi[:], nat2)
                nc.vector.tensor_sub(lo[:], nat2, hi[:])
                hiT = prolog.tile([P, QT, P], F16, tag=f"hiT_{mtag}")
                loT = prolog.tile([P, QT, P], F16, tag=f"loT_{mtag}")
                nc.sync.dma_start_transpose(hiT[:], hi[:])
                nc.sync.dma_start_transpose(loT[:], lo[:])
                nc.vector.tensor_add(dstT[:, ts], hiT[:], loT[:])

            # main first: mm1 needs all of mainT but only one c-tile of ctxT
            for q in range(NQ):
                load_transposed(main_dt, main_f32, mainT, q, "m")
            for q in range(NQ):
                load_transposed(ctx_dt, ctx_f32, ctxT, q, "c")

            # ---- main loop over c-tiles ----
            acc = psum_acc.tile([P, S], F32)  # weightedT accumulator [d, m]
            for ct in range(T):
                e_t = etile_pool.tile([P, S], F32, tag="e")
                s_part = small.tile([P, 2], F32, tag="spart")
                for h in range(2):
                    w_ps = psum_w.tile([P, HALF], F32, tag="w")
                    for j in range(0, HALF, 512):
                        w = min(512, HALF - j)
                        nc.tensor.matmul(
                            w_ps[:, j:j + w],
                            ctxT[:, ct],
                            mainT2[:, h * HALF + j: h * HALF + j + w],
                            start=True,
                            stop=True,
                        )
                    nc.scalar.activation(
                        e_t[:, h * HALF:(h + 1) * HALF],
                        w_ps[:],
                        EXP,
                        accum_out=s_part[:, h:h + 1],
                    )
                s_sum = small.tile([P, 1], F32, tag="ssum")
                nc.vector.tensor_add(s_sum[:], s_part[:, 0:1], s_part[:, 1:2])
                sinv = small.tile([P, 1], F32, tag="sinv")
                nc.vector.reciprocal(sinv[:], s_sum[:])
                ctx_s = small.tile([P, P], F32, tag="ctxs")
                nc.vector.tensor_scalar_mul(ctx_s[:], ctx_f32[:, ct], sinv[:])
                for j in range(0, S, 512):
                    w = min(512, S - j)
                    nc.tensor.matmul(
                        acc[:, j:j + w],
                        ctx_s[:],
                        e_t[:, j:j + w],
                        start=(ct == 0),
                        stop=(ct == T - 1),
                    )

            # ---- tail: evac, transpose back, subtract, store ----
            for q in range(NQ):
                ts = slice(q * QT, (q + 1) * QT)
                cs = slice(q * QW, (q + 1) * QW)
                w_hi = tailp.tile([P, QW], F16, tag="whi")
                nc.scalar.copy(w_hi[:], acc[:, cs])
                hi_nat = tailp.tile([P, QT, P], F16, tag="hinat")
                nc.sync.dma_start_transpose(hi_nat[:], w_hi[:])
                out_sb = tailp.tile([P, QT, P], F32, tag="outsb")
                nc.vector.tensor_sub(out_sb[:], main_f32[:, ts], hi_nat[:])
                if TAIL_HILO:
                    w_lo = tailp.tile([P, QW], F16, tag="wlo")
                    nc.vector.tensor_sub(w_lo[:], acc[:, cs], w_hi[:])
                    lo_nat = tailp.tile([P, QT, P], F16, tag="lonat")
                    nc.sync.dma_start_transpose(lo_nat[:], w_lo[:])
                    nc.vector.tensor_sub(out_sb[:], out_sb[:], lo_nat[:])
                nc.sync.dma_start(out_dt[:, ts], out_sb[:])

    nc.compile()
    return nc


_NC_CACHE = {}


def _get_nc(S, D):
    key = (S, D)
    if key not in _NC_CACHE:
        _NC_CACHE[key] = build_nc(S, D)
    return _NC_CACHE[key]


def kernel(context: np.ndarray, main: np.ndarray) -> np.ndarray:
    B, S, D = context.shape
    assert main.shape == (B, S, D) and B == N_CORES
    nc = _get_nc(S, D)
    in_maps = [
        {
            "context": np.ascontiguousarray(context[b], dtype=np.float32),
            "main": np.ascontiguousarray(main[b], dtype=np.float32),
        }
        for b in range(B)
    ]
    res = run_bass_kernel_spmd(nc, in_maps, list(range(N_CORES)))
    return np.stack([res.results[b]["out"] for b in range(B)], axis=0)
